# revision 1
# baseline (speedup 1.0000x reference)
"""Trainium2 Bass kernel for nn_Actor: GRU decode loop with epsilon-greedy
Gumbel-max sampling, distributed over 8 NeuronCores by vocab sharding.

Strategy
--------
All randomness in the reference depends only on step_idx, so the Gumbel noise
G = -log(-log(u)) and the epsilon-greedy draw mask are precomputed on host
with JAX CPU (bit-identical threefry). The embedding/input projection is
pre-folded on host: Epp = emb @ w_ih.T + b_ih (+ b_hh r/z parts), so the
per-step input transform becomes a row gather.

On device (per core, SPMD over 8 cores):
 - hidden state kept transposed: hT [128 part, (ktile=2, batch=32)]
 - gh = w_hh.T-chunks @ h (stationary weights), gi gathered rows transposed
   via identity matmuls, both accumulated in one PSUM bank
 - gates on ACT (sigmoid/tanh) + DVE elementwise
 - logits: col-tiled fp32 matmul (4 col groups x 2 K-tiles) into [128, 1024]
   PSUM; each core owns 4000 vocab columns (SBUF-resident w_out shard)
 - sums = logits + G (draw rows: h masked to 0 so logits=0, argmax over G
   alone, matching the reference's uniform branch); DVE max + max_index
 - per-core (val, global_idx) candidates AllGather'd across 8 cores; each
   core computes the global argmax (ties -> lowest index, like jnp.argmax)
 - winning token written to output and fed back via indirect DMA gather.
"""
import sys

sys.path.insert(0, "/opt/trn_rl_repo")

import numpy as np

B, S, V, E, H = 32, 64, 32000, 128, 256
EPS_START, EPS_END, EPS_DECAY = 0.9, 0.05, 10000.0
N_CORES = 8
VSH = V // N_CORES
GW = VSH // 4
CH = [(0, 512), (512, GW - 512)]

_CACHE = {}


def _build(n_steps):
    import concourse.bass as bass
    import concourse.mybir as mybir
    from concourse import tile, bacc

    F32 = mybir.dt.float32
    I32 = mybir.dt.int32
    U32 = mybir.dt.uint32
    AF = mybir.ActivationFunctionType
    Alu = mybir.AluOpType

    nc = bacc.Bacc(None, target_bir_lowering=False, debug=False)

    Epp = nc.dram_tensor("Epp", [V, 3 * H], F32, kind="ExternalInput")
    WoutT = nc.dram_tensor("WoutT", [2, 128, VSH], F32, kind="ExternalInput")
    WhhT = nc.dram_tensor("WhhT", [2, 128, 3 * H], F32, kind="ExternalInput")
    Gv = nc.dram_tensor("Gv", [n_steps, 128, GW], F32, kind="ExternalInput")
    dmask = nc.dram_tensor("dmask", [128, n_steps * 64], F32, kind="ExternalInput")
    goff = nc.dram_tensor("goff", [128, 1], F32, kind="ExternalInput")
    ident = nc.dram_tensor("ident", [32, 32], F32, kind="ExternalInput")
    out = nc.dram_tensor("out", [B, n_steps], I32, kind="ExternalOutput")

    with tile.TileContext(nc) as tc:
        with (
            tc.tile_pool(name="stat", bufs=1) as stat,
            tc.tile_pool(name="gbuf", bufs=3) as gbuf,
            tc.tile_pool(name="work", bufs=2) as work,
            tc.tile_pool(name="ps_log", bufs=2, space="PSUM") as ps_log,
            tc.tile_pool(name="ps_gate", bufs=2, space="PSUM") as ps_gate,
            tc.tile_pool(name="dram", bufs=2, space="DRAM") as dram,
        ):
            t_WoutT = stat.tile([128, 2, VSH], F32)
            nc.sync.dma_start(t_WoutT[:], WoutT.ap().rearrange("t p n -> p t n"))
            t_WhhT = stat.tile([128, 2, 3 * H], F32)
            nc.sync.dma_start(t_WhhT[:], WhhT.ap().rearrange("t p n -> p t n"))
            t_dmask = stat.tile([128, n_steps * 64], F32)
            nc.sync.dma_start(t_dmask[:], dmask[:])
            t_goff = stat.tile([128, 1], F32)
            nc.sync.dma_start(t_goff[:], goff[:])
            t_ident = stat.tile([32, 32], F32)
            nc.sync.dma_start(t_ident[:], ident[:])
            t_out = stat.tile([B, n_steps], I32)

            hbuf0 = stat.tile([128, 64], F32, tag="h0")
            hbuf1 = stat.tile([128, 64], F32, tag="h1")
            hbuf = [hbuf0, hbuf1]
            nc.vector.memset(hbuf[1][:], 0.0)
            t_tok0 = stat.tile([B, 1], I32)
            nc.vector.memset(t_tok0[:], 0)

            tok_prev = t_tok0

            for s in range(n_steps):
                t_G = gbuf.tile([128, GW], F32, tag="G")
                nc.sync.dma_start(t_G[:], Gv[s, :, :])

                prev = hbuf[(s + 1) % 2]
                cur = hbuf[s % 2]

                # gh accumulation (one group per PSUM bank per step)
                pg = ps_gate.tile([128, 256], F32, tag="pg")
                for c in range(6):
                    dst_off = 32 * c if c < 4 else 128 + 32 * (c - 4)
                    dst = pg[:, dst_off : dst_off + 32]
                    for k in range(2):
                        nc.tensor.matmul(
                            dst,
                            t_WhhT[:, k, 128 * c : 128 * (c + 1)],
                            prev[:, 32 * k : 32 * (k + 1)],
                            start=(c == 0 and k == 0),
                            stop=False,
                        )

                t_gi = work.tile([B, 3 * H], F32, tag="gi")
                nc.gpsimd.indirect_dma_start(
                    out=t_gi[:],
                    out_offset=None,
                    in_=Epp[:],
                    in_offset=bass.IndirectOffsetOnAxis(ap=tok_prev[:, :1], axis=0),
                )

                for c in range(6):
                    if c < 4:
                        dst = pg[:, 32 * c : 32 * (c + 1)]
                    else:
                        dst = pg[:, 192 + 32 * (c - 4) : 192 + 32 * (c - 3)]
                    nc.tensor.matmul(
                        dst,
                        t_gi[:, 128 * c : 128 * (c + 1)],
                        t_ident[:],
                        start=False,
                        stop=(c == 5),
                    )

                t_rz = work.tile([128, 128], F32, tag="rz")
                nc.scalar.activation(t_rz[:], pg[:, 0:128], AF.Sigmoid)
                t_rhn = work.tile([128, 64], F32, tag="rhn")
                nc.vector.tensor_tensor(
                    out=t_rhn[:], in0=t_rz[:, 0:64], in1=pg[:, 128:192], op=Alu.mult
                )
                t_npre = work.tile([128, 64], F32, tag="npre")
                nc.vector.tensor_tensor(
                    out=t_npre[:], in0=t_rhn[:], in1=pg[:, 192:256], op=Alu.add
                )
                t_n = work.tile([128, 64], F32, tag="n")
                nc.scalar.activation(t_n[:], t_npre[:], AF.Tanh)
                t_d = work.tile([128, 64], F32, tag="d")
                nc.vector.tensor_tensor(
                    out=t_d[:], in0=prev[:], in1=t_n[:], op=Alu.subtract
                )
                t_zd = work.tile([128, 64], F32, tag="zd")
                nc.vector.tensor_tensor(
                    out=t_zd[:], in0=t_d[:], in1=t_rz[:, 64:128], op=Alu.mult
                )
                nc.vector.tensor_tensor(out=cur[:], in0=t_zd[:], in1=t_n[:], op=Alu.add)

                t_hmm = work.tile([128, 64], F32, tag="hmm")
                nc.vector.tensor_tensor(
                    out=t_hmm[:],
                    in0=cur[:],
                    in1=t_dmask[:, 64 * s : 64 * (s + 1)],
                    op=Alu.mult,
                )

                pl = ps_log.tile([128, 1024], F32, tag="pl")
                for g in range(4):
                    for (coff, cw) in CH:
                        for k in range(2):
                            nc.tensor.matmul(
                                pl[32 * g : 32 * (g + 1), coff : coff + cw],
                                t_hmm[:, 32 * k : 32 * (k + 1)],
                                t_WoutT[:, k, g * GW + coff : g * GW + coff + cw],
                                start=(k == 0),
                                stop=(k == 1),
                                tile_position=(0, 32 * g),
                            )

                t_sums = work.tile([128, GW], F32, tag="sums")
                nc.vector.tensor_tensor(
                    out=t_sums[:], in0=pl[:, 0:GW], in1=t_G[:], op=Alu.add
                )
                t_top8 = work.tile([128, 8], F32, tag="top8")
                nc.vector.max(out=t_top8[:], in_=t_sums[:])
                t_idx8 = work.tile([128, 8], U32, tag="idx8")
                nc.vector.max_index(t_idx8[:], t_top8[:], t_sums[:])

                t_idxf = work.tile([128, 1], F32, tag="idxf")
                nc.vector.tensor_copy(t_idxf[:], t_idx8[:, 0:1])
                t_gidx = work.tile([128, 1], F32, tag="gidx")
                nc.vector.tensor_tensor(
                    out=t_gidx[:], in0=t_idxf[:], in1=t_goff[:], op=Alu.add
                )
                cc_in = dram.tile([128, 2], F32, tag="ccin")
                nc.gpsimd.dma_start(cc_in[:, 0:1], t_top8[:, 0:1])
                nc.gpsimd.dma_start(cc_in[:, 1:2], t_gidx[:])
                cc_out = dram.tile([128 * N_CORES, 2], F32, tag="ccout")
                nc.gpsimd.collective_compute(
                    "AllGather",
                    Alu.bypass,
                    replica_groups=[list(range(N_CORES))],
                    ins=[cc_in.opt()],
                    outs=[cc_out.opt()],
                )

                t_all = work.tile([B, 64], F32, tag="all")
                nc.sync.dma_start(
                    t_all[:].rearrange("b (v c g) -> b v c g", v=2, c=N_CORES),
                    cc_out[:].rearrange("(c g b) v -> b v c g", c=N_CORES, g=4),
                )
                t_m = work.tile([B, 1], F32, tag="m")
                nc.vector.tensor_reduce(
                    out=t_m[:], in_=t_all[:, 0:32], axis=mybir.AxisListType.X, op=Alu.max
                )
                t_msk = work.tile([B, 32], F32, tag="msk")
                nc.vector.tensor_scalar(
                    out=t_msk[:],
                    in0=t_all[:, 0:32],
                    scalar1=t_m[:],
                    scalar2=1e9,
                    op0=Alu.not_equal,
                    op1=Alu.mult,
                )
                t_mi = work.tile([B, 32], F32, tag="mi")
                nc.vector.tensor_tensor(
                    out=t_mi[:], in0=t_msk[:], in1=t_all[:, 32:64], op=Alu.add
                )
                t_widx = work.tile([B, 1], F32, tag="widx")
                nc.vector.tensor_reduce(
                    out=t_widx[:], in_=t_mi[:], axis=mybir.AxisListType.X, op=Alu.min
                )
                t_tok = work.tile([B, 1], I32, tag="tok")
                nc.vector.tensor_copy(t_tok[:], t_widx[:])
                nc.vector.tensor_copy(t_out[:, s : s + 1], t_tok[:])
                tok_prev = t_tok

            nc.sync.dma_start(out[:], t_out[:])

    nc.compile()
    return nc


def _precompute(inputs, n_steps):
    import jax
    import jax.numpy as jnp

    cpu = jax.devices("cpu")[0]
    with jax.default_device(cpu):
        emb = jnp.asarray(np.asarray(inputs["embedding"]), jnp.float32)
        w_ih = jnp.asarray(np.asarray(inputs["w_ih"]), jnp.float32)
        b_ih = jnp.asarray(np.asarray(inputs["b_ih"]), jnp.float32)
        b_hh = jnp.asarray(np.asarray(inputs["b_hh"]), jnp.float32)
        b_out = jnp.asarray(np.asarray(inputs["b_out"]), jnp.float32)

        Epp = jnp.dot(emb, w_ih.T) + b_ih
        bhh_rz = jnp.concatenate([b_hh[: 2 * H], jnp.zeros((H,), jnp.float32)])
        Epp = np.asarray(Epp + bhh_rz, np.float32)
        assert np.all(np.asarray(b_hh[2 * H :]) == 0.0), "b_hh n-part must be zero"

        key = jax.random.key(42)
        steps = jnp.arange(n_steps)
        draws = np.zeros((n_steps, B), bool)
        G_full = np.zeros((n_steps, B, V), np.float32)
        for s in range(n_steps):
            si = steps[s]
            eps = EPS_END + (EPS_START - EPS_END) * jnp.exp(
                -4.0 * si.astype(jnp.float32) / EPS_DECAY
            )
            k = jax.random.fold_in(key, si)
            k1, k2 = jax.random.split(k)
            draws[s] = np.asarray(jax.random.uniform(k1, (B,)) <= eps)
            u = jax.random.uniform(k2, (B, V), minval=1e-12, maxval=1.0)
            G_full[s] = np.asarray(-jnp.log(-jnp.log(u)))
        G_full = G_full + np.asarray(b_out, np.float32)[None, None, :]

    w_hh = np.asarray(inputs["w_hh"], np.float32)
    w_out = np.asarray(inputs["w_out"], np.float32)
    WhhT = np.ascontiguousarray(w_hh.T).reshape(2, 128, 3 * H)

    dmask = np.zeros((n_steps, 2, B), np.float32)
    dmask[:, :, :] = (~draws)[:, None, :].astype(np.float32)
    dmask_dev = np.broadcast_to(
        dmask.reshape(1, n_steps * 64), (128, n_steps * 64)
    ).copy()
    ident = np.eye(32, dtype=np.float32)

    in_maps = []
    for c in range(N_CORES):
        WoutT_c = np.ascontiguousarray(w_out[c * VSH : (c + 1) * VSH].T).reshape(
            2, 128, VSH
        )
        Gc = G_full[:, :, c * VSH : (c + 1) * VSH].reshape(n_steps, B, 4, GW)
        Gc = np.ascontiguousarray(Gc.transpose(0, 2, 1, 3)).reshape(n_steps, 128, GW)
        goff = np.zeros((128, 1), np.float32)
        for g in range(4):
            goff[32 * g : 32 * (g + 1)] = c * VSH + g * GW
        in_maps.append(
            dict(
                Epp=Epp,
                WoutT=WoutT_c,
                WhhT=WhhT,
                Gv=Gc,
                dmask=dmask_dev,
                goff=goff,
                ident=ident,
            )
        )
    return in_maps


def kernel(**inputs) -> np.ndarray:
    from concourse.bass_utils import run_bass_kernel_spmd

    if "nc" not in _CACHE:
        _CACHE["nc"] = _build(S)
    in_maps = _precompute(inputs, S)
    res = run_bass_kernel_spmd(_CACHE["nc"], in_maps, core_ids=list(range(N_CORES)))
    return np.asarray(res.results[0]["out"]).astype(np.int32)


# revision 4
# speedup vs baseline: 1.9149x; 1.9149x over previous
"""Trainium2 Bass kernel for nn_Actor: GRU decode loop with epsilon-greedy
Gumbel-max sampling, distributed over 8 NeuronCores by vocab sharding.

Strategy
--------
All randomness in the reference depends only on step_idx, so the Gumbel noise
G = -log(-log(u)) and the epsilon-greedy draw mask are precomputed on host
with JAX CPU (bit-identical threefry). The embedding/input projection is
pre-folded on host: Epp = emb @ w_ih.T + b_ih (+ b_hh r/z parts), so the
per-step input transform becomes a row gather.

On device (per core, SPMD over 8 cores):
 - hidden state kept transposed: hT [128 part, (ktile=2, batch=32)]
 - gh = w_hh.T-chunks @ h (stationary weights), gi gathered rows transposed
   via identity matmuls, both accumulated in one PSUM bank
 - gates on ACT (sigmoid/tanh) + DVE elementwise
 - logits: col-tiled fp32 matmul (4 col groups x 2 K-tiles) into [128, 1024]
   PSUM; each core owns 4000 vocab columns (SBUF-resident w_out shard)
 - sums = logits + G (draw rows: h masked to 0 so logits=0, argmax over G
   alone, matching the reference's uniform branch); DVE max + max_index
 - per-core (val, global_idx) candidates AllGather'd across 8 cores; each
   core computes the global argmax (ties -> lowest index, like jnp.argmax)
 - winning token written to output and fed back via indirect DMA gather.

Performance notes (measured by long-unroll differencing on silicon):
 - col-tiled fp32 matmul blocks run ~2.7 us (concurrent column groups give
   ~3.7x over flat tiling; fp32's internal 2-pass penalty hides under the
   cross-group overlap, matching bf16 speed)
 - the per-step cost is dominated by the serial dependency chain's
   fixed latencies (DMA round-trips and cross-engine semaphore hops),
   not compute; isolated microbenchmarks showed ACT/gpsimd-issued DMAs
   with lower chain latency than sync/HWDGE, but switching the exchange
   DMAs to those queues regressed end-to-end (queue head-of-line with
   the activations), so chain DMAs stay on sync/gpsimd
 - the small-message AllGather itself is ~1 us; a split-batch two-stream
   software pipeline was tried and abandoned (in-order engine queues
   serialize cross-stream work; measured slower than the single chain).
"""
import sys

sys.path.insert(0, "/opt/trn_rl_repo")

import numpy as np

B, S, V, E, H = 32, 64, 32000, 128, 256
EPS_START, EPS_END, EPS_DECAY = 0.9, 0.05, 10000.0
N_CORES = 8
VSH = V // N_CORES
GW = VSH // 4
CH = [(0, 512), (512, GW - 512)]

_CACHE = {}


def _build(n_steps):
    import concourse.bass as bass
    import concourse.mybir as mybir
    from concourse import tile, bacc

    F32 = mybir.dt.float32
    I32 = mybir.dt.int32
    U32 = mybir.dt.uint32
    AF = mybir.ActivationFunctionType
    Alu = mybir.AluOpType

    nc = bacc.Bacc(None, target_bir_lowering=False, debug=False)

    Epp = nc.dram_tensor("Epp", [V, 3 * H], F32, kind="ExternalInput")
    WoutT = nc.dram_tensor("WoutT", [2, 128, VSH], F32, kind="ExternalInput")
    WhhT = nc.dram_tensor("WhhT", [2, 128, 3 * H], F32, kind="ExternalInput")
    Gv = nc.dram_tensor("Gv", [n_steps, 128, GW], F32, kind="ExternalInput")
    dmask = nc.dram_tensor("dmask", [128, n_steps * 64], F32, kind="ExternalInput")
    goff = nc.dram_tensor("goff", [128, 1], F32, kind="ExternalInput")
    ident = nc.dram_tensor("ident", [32, 32], F32, kind="ExternalInput")
    out = nc.dram_tensor("out", [B, n_steps], I32, kind="ExternalOutput")

    with tile.TileContext(nc) as tc:
        with (
            tc.tile_pool(name="stat", bufs=1) as stat,
            tc.tile_pool(name="gbuf", bufs=3) as gbuf,
            tc.tile_pool(name="work", bufs=2) as work,
            tc.tile_pool(name="ps_log", bufs=2, space="PSUM") as ps_log,
            tc.tile_pool(name="ps_gate", bufs=2, space="PSUM") as ps_gate,
            tc.tile_pool(name="dram", bufs=2, space="DRAM") as dram,
        ):
            t_WoutT = stat.tile([128, 2, VSH], F32)
            nc.sync.dma_start(t_WoutT[:], WoutT.ap().rearrange("t p n -> p t n"))
            t_WhhT = stat.tile([128, 2, 3 * H], F32)
            nc.sync.dma_start(t_WhhT[:], WhhT.ap().rearrange("t p n -> p t n"))
            t_dmask = stat.tile([128, n_steps * 64], F32)
            nc.sync.dma_start(t_dmask[:], dmask[:])
            t_goff = stat.tile([128, 1], F32)
            nc.sync.dma_start(t_goff[:], goff[:])
            t_ident = stat.tile([32, 32], F32)
            nc.sync.dma_start(t_ident[:], ident[:])
            t_out = stat.tile([B, n_steps], I32)

            hbuf0 = stat.tile([128, 64], F32, tag="h0")
            hbuf1 = stat.tile([128, 64], F32, tag="h1")
            hbuf = [hbuf0, hbuf1]
            nc.vector.memset(hbuf[1][:], 0.0)
            t_tok0 = stat.tile([B, 1], I32)
            nc.vector.memset(t_tok0[:], 0)

            tok_prev = t_tok0

            for s in range(n_steps):
                t_G = gbuf.tile([128, GW], F32, tag="G")
                nc.sync.dma_start(t_G[:], Gv[s, :, :])

                prev = hbuf[(s + 1) % 2]
                cur = hbuf[s % 2]

                # gh accumulation (one group per PSUM bank per step)
                pg = ps_gate.tile([128, 256], F32, tag="pg")
                for c in range(6):
                    dst_off = 32 * c if c < 4 else 128 + 32 * (c - 4)
                    dst = pg[:, dst_off : dst_off + 32]
                    for k in range(2):
                        nc.tensor.matmul(
                            dst,
                            t_WhhT[:, k, 128 * c : 128 * (c + 1)],
                            prev[:, 32 * k : 32 * (k + 1)],
                            start=(c == 0 and k == 0),
                            stop=False,
                        )

                t_gi = work.tile([B, 3 * H], F32, tag="gi")
                nc.gpsimd.indirect_dma_start(
                    out=t_gi[:],
                    out_offset=None,
                    in_=Epp[:],
                    in_offset=bass.IndirectOffsetOnAxis(ap=tok_prev[:, :1], axis=0),
                )

                for c in range(6):
                    if c < 4:
                        dst = pg[:, 32 * c : 32 * (c + 1)]
                    else:
                        dst = pg[:, 192 + 32 * (c - 4) : 192 + 32 * (c - 3)]
                    nc.tensor.matmul(
                        dst,
                        t_gi[:, 128 * c : 128 * (c + 1)],
                        t_ident[:],
                        start=False,
                        stop=(c == 5),
                    )

                t_rz = work.tile([128, 128], F32, tag="rz")
                nc.scalar.activation(t_rz[:], pg[:, 0:128], AF.Sigmoid)
                t_rhn = work.tile([128, 64], F32, tag="rhn")
                nc.vector.tensor_tensor(
                    out=t_rhn[:], in0=t_rz[:, 0:64], in1=pg[:, 128:192], op=Alu.mult
                )
                t_npre = work.tile([128, 64], F32, tag="npre")
                nc.vector.tensor_tensor(
                    out=t_npre[:], in0=t_rhn[:], in1=pg[:, 192:256], op=Alu.add
                )
                t_n = work.tile([128, 64], F32, tag="n")
                nc.scalar.activation(t_n[:], t_npre[:], AF.Tanh)
                t_d = work.tile([128, 64], F32, tag="d")
                nc.vector.tensor_tensor(
                    out=t_d[:], in0=prev[:], in1=t_n[:], op=Alu.subtract
                )
                t_zd = work.tile([128, 64], F32, tag="zd")
                nc.vector.tensor_tensor(
                    out=t_zd[:], in0=t_d[:], in1=t_rz[:, 64:128], op=Alu.mult
                )
                nc.vector.tensor_tensor(out=cur[:], in0=t_zd[:], in1=t_n[:], op=Alu.add)

                t_hmm = work.tile([128, 64], F32, tag="hmm")
                nc.vector.tensor_tensor(
                    out=t_hmm[:],
                    in0=cur[:],
                    in1=t_dmask[:, 64 * s : 64 * (s + 1)],
                    op=Alu.mult,
                )

                pl = ps_log.tile([128, 1024], F32, tag="pl")
                for g in range(4):
                    for (coff, cw) in CH:
                        for k in range(2):
                            nc.tensor.matmul(
                                pl[32 * g : 32 * (g + 1), coff : coff + cw],
                                t_hmm[:, 32 * k : 32 * (k + 1)],
                                t_WoutT[:, k, g * GW + coff : g * GW + coff + cw],
                                start=(k == 0),
                                stop=(k == 1),
                                tile_position=(0, 32 * g),
                            )

                t_sums = work.tile([128, GW], F32, tag="sums")
                nc.vector.tensor_tensor(
                    out=t_sums[:], in0=pl[:, 0:GW], in1=t_G[:], op=Alu.add
                )
                t_top8 = work.tile([128, 8], F32, tag="top8")
                nc.vector.max(out=t_top8[:], in_=t_sums[:])
                t_idx8 = work.tile([128, 8], U32, tag="idx8")
                nc.vector.max_index(t_idx8[:], t_top8[:], t_sums[:])

                t_idxf = work.tile([128, 1], F32, tag="idxf")
                nc.vector.tensor_copy(t_idxf[:], t_idx8[:, 0:1])
                t_gidx = work.tile([128, 1], F32, tag="gidx")
                nc.vector.tensor_tensor(
                    out=t_gidx[:], in0=t_idxf[:], in1=t_goff[:], op=Alu.add
                )
                cc_in = dram.tile([128, 2], F32, tag="ccin")
                nc.gpsimd.dma_start(cc_in[:, 0:1], t_top8[:, 0:1])
                nc.gpsimd.dma_start(cc_in[:, 1:2], t_gidx[:])
                cc_out = dram.tile([128 * N_CORES, 2], F32, tag="ccout")
                nc.gpsimd.collective_compute(
                    "AllGather",
                    Alu.bypass,
                    replica_groups=[list(range(N_CORES))],
                    ins=[cc_in.opt()],
                    outs=[cc_out.opt()],
                )

                t_all = work.tile([B, 64], F32, tag="all")
                nc.sync.dma_start(
                    t_all[:].rearrange("b (v c g) -> b v c g", v=2, c=N_CORES),
                    cc_out[:].rearrange("(c g b) v -> b v c g", c=N_CORES, g=4),
                )
                t_m = work.tile([B, 1], F32, tag="m")
                nc.vector.tensor_reduce(
                    out=t_m[:], in_=t_all[:, 0:32], axis=mybir.AxisListType.X, op=Alu.max
                )
                t_msk = work.tile([B, 32], F32, tag="msk")
                nc.vector.tensor_scalar(
                    out=t_msk[:],
                    in0=t_all[:, 0:32],
                    scalar1=t_m[:],
                    scalar2=1e9,
                    op0=Alu.not_equal,
                    op1=Alu.mult,
                )
                t_mi = work.tile([B, 32], F32, tag="mi")
                nc.vector.tensor_tensor(
                    out=t_mi[:], in0=t_msk[:], in1=t_all[:, 32:64], op=Alu.add
                )
                t_widx = work.tile([B, 1], F32, tag="widx")
                nc.vector.tensor_reduce(
                    out=t_widx[:], in_=t_mi[:], axis=mybir.AxisListType.X, op=Alu.min
                )
                t_tok = work.tile([B, 1], I32, tag="tok")
                nc.vector.tensor_copy(t_tok[:], t_widx[:])
                nc.vector.tensor_copy(t_out[:, s : s + 1], t_tok[:])
                tok_prev = t_tok

            nc.sync.dma_start(out[:], t_out[:])

    nc.compile()
    return nc


def _precompute(inputs, n_steps):
    import jax
    import jax.numpy as jnp

    cpu = jax.devices("cpu")[0]
    with jax.default_device(cpu):
        emb = jnp.asarray(np.asarray(inputs["embedding"]), jnp.float32)
        w_ih = jnp.asarray(np.asarray(inputs["w_ih"]), jnp.float32)
        b_ih = jnp.asarray(np.asarray(inputs["b_ih"]), jnp.float32)
        b_hh = jnp.asarray(np.asarray(inputs["b_hh"]), jnp.float32)
        b_out = jnp.asarray(np.asarray(inputs["b_out"]), jnp.float32)

        Epp = jnp.dot(emb, w_ih.T) + b_ih
        bhh_rz = jnp.concatenate([b_hh[: 2 * H], jnp.zeros((H,), jnp.float32)])
        Epp = np.asarray(Epp + bhh_rz, np.float32)
        assert np.all(np.asarray(b_hh[2 * H :]) == 0.0), "b_hh n-part must be zero"

        key = jax.random.key(42)
        steps = jnp.arange(n_steps)
        draws = np.zeros((n_steps, B), bool)
        G_full = np.zeros((n_steps, B, V), np.float32)
        for s in range(n_steps):
            si = steps[s]
            eps = EPS_END + (EPS_START - EPS_END) * jnp.exp(
                -4.0 * si.astype(jnp.float32) / EPS_DECAY
            )
            k = jax.random.fold_in(key, si)
            k1, k2 = jax.random.split(k)
            draws[s] = np.asarray(jax.random.uniform(k1, (B,)) <= eps)
            u = jax.random.uniform(k2, (B, V), minval=1e-12, maxval=1.0)
            G_full[s] = np.asarray(-jnp.log(-jnp.log(u)))
        G_full = G_full + np.asarray(b_out, np.float32)[None, None, :]

    w_hh = np.asarray(inputs["w_hh"], np.float32)
    w_out = np.asarray(inputs["w_out"], np.float32)
    WhhT = np.ascontiguousarray(w_hh.T).reshape(2, 128, 3 * H)

    dmask = np.zeros((n_steps, 2, B), np.float32)
    dmask[:, :, :] = (~draws)[:, None, :].astype(np.float32)
    dmask_dev = np.broadcast_to(
        dmask.reshape(1, n_steps * 64), (128, n_steps * 64)
    ).copy()
    ident = np.eye(32, dtype=np.float32)

    in_maps = []
    for c in range(N_CORES):
        WoutT_c = np.ascontiguousarray(w_out[c * VSH : (c + 1) * VSH].T).reshape(
            2, 128, VSH
        )
        Gc = G_full[:, :, c * VSH : (c + 1) * VSH].reshape(n_steps, B, 4, GW)
        Gc = np.ascontiguousarray(Gc.transpose(0, 2, 1, 3)).reshape(n_steps, 128, GW)
        goff = np.zeros((128, 1), np.float32)
        for g in range(4):
            goff[32 * g : 32 * (g + 1)] = c * VSH + g * GW
        in_maps.append(
            dict(
                Epp=Epp,
                WoutT=WoutT_c,
                WhhT=WhhT,
                Gv=Gc,
                dmask=dmask_dev,
                goff=goff,
                ident=ident,
            )
        )
    return in_maps


def kernel(**inputs) -> np.ndarray:
    from concourse.bass_utils import run_bass_kernel_spmd

    if "nc" not in _CACHE:
        _CACHE["nc"] = _build(S)
    in_maps = _precompute(inputs, S)
    res = run_bass_kernel_spmd(_CACHE["nc"], in_maps, core_ids=list(range(N_CORES)))
    return np.asarray(res.results[0]["out"]).astype(np.int32)
